# revision 19
# baseline (speedup 1.0000x reference)
"""Trainium2 Bass kernel for the GRU decoder problem.

Strategy (fp8 DoubleRow, 4-engine pipeline)
-------------------------------------------
Data-parallel over 8 NeuronCores: batch 8192 -> 1024 per core; weights
replicated.  Feature-major on-chip layout (features on SBUF partitions,
batch on the free dim); all recurrent matmuls run in fp8(e4m3) with
perf_mode=DoubleRow (256-contraction per instruction, 2x PE throughput).

Math (per core, B=1024, exact reformulation of the reference):
    h0   = (fc2 o fc1)(latent)            one fused [128,512] fp16 matmul
    per step t = 0..118:
        rz   = h @ G_rz + b               G_rz = Whh_rz + C@Wih_rz (t>=1)
        r,z  = sigmoid(rz)
        n    = tanh(x @ Wih_n + b_ihn + r*(h @ Whh_n + b_hhn))
        h    = n + z*(h - n)
        pred = h @ C + c_b                C = h1_w^T @ h2_w^T
        y[:, t+1] = pred;  x = pred

Precision / engine assignment (all verified rel_l2 ~ 6e-3 vs fp32):
  * fp8 e4m3 matmul operands, weights pre-scaled x8 so PSUM = 8x the
    true value; sigmoid/tanh/pred apply scale=2^-3.  h is kept in fp16
    (true scale) plus an fp8 matmul copy written directly by the DVE.
  * biases fold into the matmuls as fp8 hi/lo pairs (lo = 16x the hi
    rounding residual) against constant (8, 1/2) rows, so the
    two-feature-tile-merged sigmoids need no per-tile bias.
  * the "+ gxn" add of the n-gate runs on the Tensor engine: an fp16
    identity matmul accumulates u = r*(h@Whh_n+b) into the gxn PSUM
    bank, so tanh reads one PSUM tile and the DVE saves a full-width
    PSUM-rate op per pair.
  * elementwise is plain tensor_tensor (DVE 2x fp16 mode; the
    scalar_tensor_tensor form has NO fast mode so scales are folded
    into activation-scale/weights instead).  Per-pair chains: sigma
    order (r01, z01, r23, z23) lets pair0's d/e/h8' finish early and
    unlock the next step's block-0 matmuls.
  * the fp16 h' copy (only needed by the NEXT step's d) is computed
    on GpSimd, off the recurrence path; the x -> fp8 copy uses a
    gpsimd casting DMA; y is staged and stored as fp16 (host upcasts)
    via SP-engine HWDGE so the Pool queue stays short.
"""

import sys

import numpy as np
import ml_dtypes

if "/opt/trn_rl_repo" not in sys.path:
    sys.path.insert(0, "/opt/trn_rl_repo")

from contextlib import ExitStack

import concourse.bass as bass
import concourse.tile as tile
from concourse import bacc, mybir
from concourse.bass import ds, ts
from concourse.bass_utils import run_bass_kernel_spmd
from concourse.masks import make_identity

P = 128
H = 512
HK = 4            # H // 128 feature tiles
A = 40            # alphabet
AT = 48           # transpose-padded alphabet rows
B = 1024          # batch per core
NCOL = 512        # batch chunk (matmul N / PSUM bank)
NB = 2            # chunks per core
T = 119           # recurrent steps
SG = 17           # steps per output DMA group
NG = 7            # groups (7*17 = 119)
N_CORES = 8

SCL = 0.125       # PSUM descale (weights x8)

F16 = mybir.dt.float16
F32 = mybir.dt.float32
F8 = mybir.dt.float8e4
DR = mybir.MatmulPerfMode.DoubleRow
NPF8 = ml_dtypes.float8_e4m3

AF = mybir.ActivationFunctionType
OP = mybir.AluOpType


def _emit(nc, bench=False, repeat=1):
    # ---- DRAM I/O ------------------------------------------------------
    d_latT = nc.dram_tensor("latT", [P, B], F16, kind="ExternalInput").ap()
    d_hemat = nc.dram_tensor("hemat", [P, H], F16, kind="ExternalInput").ap()
    d_fb = nc.dram_tensor("fb", [P, HK], F32, kind="ExternalInput").ap()
    d_grz8 = nc.dram_tensor("grz8", [P, HK, 2 * H], F8, kind="ExternalInput").ap()
    d_whhrz8 = nc.dram_tensor("whhrz8", [P, HK, 2 * H], F8, kind="ExternalInput").ap()
    d_whhn8 = nc.dram_tensor("whhn8", [P, HK, H], F8, kind="ExternalInput").ap()
    d_cmat8 = nc.dram_tensor("cmat8", [P, HK, AT], F8, kind="ExternalInput").ap()
    d_wihn8 = nc.dram_tensor("wihn8", [64, 4, H], F8, kind="ExternalInput").ap()
    d_wihn0 = nc.dram_tensor("wihn0", [64, 4, H], F8, kind="ExternalInput").ap()
    d_brzf = nc.dram_tensor("brzf", [32, 2, 2 * H], F8, kind="ExternalInput").ap()
    d_brz0 = nc.dram_tensor("brz0", [32, 2, 2 * H], F8, kind="ExternalInput").ap()
    d_bhhn = nc.dram_tensor("bhhn", [32, 2, H], F8, kind="ExternalInput").ap()
    d_cb = nc.dram_tensor("cb", [A, 1], F32, kind="ExternalInput").ap()
    if bench:
        d_y = nc.dram_tensor("ybench", [B, T, A], F16, kind="Internal").ap()
        d_dummy = nc.dram_tensor("y", [A, 1], F32, kind="ExternalOutput").ap()
    else:
        d_y = nc.dram_tensor("y", [B, T, A], F16, kind="ExternalOutput").ap()

    with tile.TileContext(nc) as tc, ExitStack() as ctx:
        const = ctx.enter_context(tc.tile_pool(name="const", bufs=1))
        state = ctx.enter_context(tc.tile_pool(name="state", bufs=1))
        ew = ctx.enter_context(tc.tile_pool(name="ew", bufs=2))
        stg = ctx.enter_context(tc.tile_pool(name="stg", bufs=2))
        # PSUM: rz pairs 2x2 banks + gates/pred pairs 2x2 banks = 8 banks
        rz_ps = ctx.enter_context(tc.tile_pool(name="rz_ps", bufs=2, space="PSUM"))
        gt_ps = ctx.enter_context(tc.tile_pool(name="gt_ps", bufs=2, space="PSUM"))

        # ---- load constants/weights ------------------------------------
        def cload(name, dram, shape, dtype):
            t = const.tile(shape, dtype, tag=name, name=name)
            nc.sync.dma_start(t[:], dram)
            return t

        latT = cload("latT", d_latT, [P, B], F16)
        hemat = cload("hemat", d_hemat, [P, H], F16)
        fb = cload("fb", d_fb, [P, HK], F32)
        grz8 = cload("grz8", d_grz8, [P, HK, 2 * H], F8)
        whhrz8 = cload("whhrz8", d_whhrz8, [P, HK, 2 * H], F8)
        whhn8 = cload("whhn8", d_whhn8, [P, HK, H], F8)
        cmat8 = cload("cmat8", d_cmat8, [P, HK, AT], F8)
        wihn8 = cload("wihn8", d_wihn8, [64, 4, H], F8)
        wihn0 = cload("wihn0", d_wihn0, [64, 4, H], F8)
        brzf = cload("brzf", d_brzf, [32, 2, 2 * H], F8)
        brz0 = cload("brz0", d_brz0, [32, 2, 2 * H], F8)
        bhhn = cload("bhhn", d_bhhn, [32, 2, H], F8)
        cb = cload("cb", d_cb, [A, 1], F32)

        # constant rows for the bias matmuls: partition 0 = (8, 1/2)
        ones32 = const.tile([32, 2, NCOL], F8, tag="ones32", name="ones32")
        nc.gpsimd.memset(ones32[:], 0.0)
        nc.gpsimd.memset(ones32[0:1, 0, :], 8.0)
        nc.gpsimd.memset(ones32[0:1, 1, :], 0.5)

        # fp16 identity for the PE-side "accumulate u into gxn PSUM" add
        ident = const.tile([P, P], F16, tag="ident", name="ident")
        make_identity(nc, ident[:])

        # ---- persistent state ------------------------------------------
        h8 = [
            [state.tile([P, HK, NCOL], F8, tag=f"h8_{p}{c}", name=f"h8_{p}{c}")
             for c in range(NB)]
            for p in range(2)
        ]
        # x8 [64, 4, NCOL]: slots 0,1 = x data (rows 0..39 = pred, cast
        # per step); slots 2,3 = (8, 1/2) const rows on partition 0 for
        # the b_ihn / gxn0 bias folds.
        x8 = [
            [state.tile([64, 4, NCOL], F8, tag=f"x8_{p}{c}", name=f"x8_{p}{c}")
             for c in range(NB)]
            for p in range(2)
        ]
        for p in range(2):
            for c in range(NB):
                nc.gpsimd.memset(x8[p][c][:], 0.0)
                nc.gpsimd.memset(x8[p][c][0:1, 2, :], 8.0)
                nc.gpsimd.memset(x8[p][c][0:1, 3, :], 0.5)
        pred16 = [state.tile([P, NCOL], F16, tag=f"pr{c}", name=f"pr{c}")
                  for c in range(NB)]
        for c in range(NB):
            nc.gpsimd.memset(pred16[c][:], 0.0)

        # ---- h0 = fused-fc(latent), fp16 -------------------------------
        for c in range(NB):
            for pair in range(2):
                pt = gt_ps.tile([P, 2, NCOL], F32, tag="g", name="h0ps")
                for i in range(2):
                    m = 2 * pair + i
                    nc.tensor.matmul(
                        pt[:, i, :], hemat[:, ts(m, P)], latT[:, ts(c, NCOL)],
                        start=True, stop=True,
                    )
                for i in range(2):
                    m = 2 * pair + i
                    nc.scalar.activation(
                        h8[0][c][:, m, :], pt[:, i, :], AF.Identity,
                        bias=fb[:, m : m + 1],
                    )

        stage = [None]

        def mm1(t, c, half):
            """rz DR matmuls + sigmoids for sigma-pairs [2*half, 2*half+1].

            sigma order over halves: (r01, z01) then (r23, z23), so each
            pair's r and z are both ready early for its elementwise chain.
            """
            par = t % 2
            h8r = h8[par][c]
            w_rz = grz8 if t else whhrz8
            b_blk = brzf if t else brz0
            if half == 0:
                st[(c, "r")] = ew.tile([P, HK, NCOL], F16, tag=f"r{c}", name="r")
                st[(c, "z")] = ew.tile([P, HK, NCOL], F16, tag=f"z{c}", name="z")
            for gate, pair in ((0, half), (1, half)):
                pt = rz_ps.tile([P, 2, NCOL], F32, tag="rz", name="rz")
                for i in range(2):
                    mp = 4 * gate + 2 * pair + i
                    for b in range(2):
                        nc.tensor.matmul(
                            pt[:, i, :],
                            w_rz[:, 2 * b : 2 * b + 2, ts(mp, P)],
                            h8r[:, 2 * b : 2 * b + 2, :],
                            start=(b == 0), stop=False, perf_mode=DR,
                        )
                    nc.tensor.matmul(
                        pt[:, i, :], b_blk[:, :, ts(mp, P)], ones32[:],
                        start=False, stop=True, perf_mode=DR,
                    )
                dst = st[(c, "r")] if gate == 0 else st[(c, "z")]
                q = pair * 2
                nc.scalar.activation(dst[:, q : q + 2, :], pt[:], AF.Sigmoid,
                                     scale=SCL)

        def mm2(t, c):
            """ghn/gxn DR matmuls + u = r*ghn (DVE)."""
            par = t % 2
            h8r = h8[par][c]
            x8r = x8[par][c]
            w_xn = wihn8 if t else wihn0
            r16 = st[(c, "r")]
            u16 = ew.tile([P, HK, NCOL], F16, tag=f"u{c}", name="u")
            pxs = []
            for pair in range(2):
                pg = gt_ps.tile([P, 2, NCOL], F32, tag="g", name="pg")
                for i in range(2):
                    m = 2 * pair + i
                    for b in range(2):
                        nc.tensor.matmul(
                            pg[:, i, :],
                            whhn8[:, 2 * b : 2 * b + 2, ts(m, P)],
                            h8r[:, 2 * b : 2 * b + 2, :],
                            start=(b == 0), stop=False, perf_mode=DR,
                        )
                    nc.tensor.matmul(
                        pg[:, i, :], bhhn[:, :, ts(m, P)], ones32[:],
                        start=False, stop=True, perf_mode=DR,
                    )
                px = rz_ps.tile([P, 2, NCOL], F32, tag="rz", name="px")
                for i in range(2):
                    m = 2 * pair + i
                    nc.tensor.matmul(
                        px[:, i, :], w_xn[:, 0:2, ts(m, P)], x8r[:, 0:2, :],
                        start=True, stop=False, perf_mode=DR,
                    )
                    nc.tensor.matmul(
                        px[:, i, :], w_xn[:, 2:4, ts(m, P)], x8r[:, 2:4, :],
                        start=False, stop=False, perf_mode=DR,
                    )
                pxs.append(px)
                q = 2 * pair
                nc.vector.tensor_tensor(
                    u16[:, q : q + 2, :], pg[:], r16[:, q : q + 2, :], OP.mult)
            st[(c, "u")] = u16
            st[(c, "px")] = pxs

        def tail(t, c):
            """identity-accumulate + tanh + h update (pair-wise chains).

            h is carried entirely in fp8: d = h8 - n runs on the DVE (one
            1-byte operand drops it to 1x rate but frees GpSimd from the
            big fp16 h' op); pair0's e/h8' run on GpSimd, pair1's on DVE.
            """
            par = t % 2
            h8r = h8[par][c]
            h8w = h8[1 - par][c]
            u16 = st[(c, "u")]
            z16 = st[(c, "z")]
            pxs = st[(c, "px")]
            nt = ew.tile([P, HK, NCOL], F16, tag=f"n{c}", name="n")
            d16 = ew.tile([P, HK, NCOL], F16, tag=f"d{c}", name="d")
            e16 = ew.tile([P, HK, NCOL], F16, tag=f"e{c}", name="e")
            for pair in range(2):
                px = pxs[pair]
                q = 2 * pair
                sl = (slice(None), slice(q, q + 2), slice(None))
                for i in range(2):
                    nc.tensor.matmul(
                        px[:, i, :], ident[:], u16[:, q + i, :],
                        start=False, stop=(i == 1),
                    )
                nc.scalar.activation(nt[sl], px[:], AF.Tanh, scale=SCL)
                nc.vector.tensor_tensor(d16[sl], h8r[sl], nt[sl], OP.subtract)
                if pair == 0:
                    nc.gpsimd.tensor_tensor(e16[sl], d16[sl], z16[sl], OP.mult)
                    nc.gpsimd.tensor_tensor(h8w[sl], nt[sl], e16[sl], OP.add)
                else:
                    nc.vector.tensor_tensor(e16[sl], d16[sl], z16[sl], OP.mult)
                    nc.vector.tensor_tensor(h8w[sl], nt[sl], e16[sl], OP.add)

        def emit_pred(t, c):
            """pred = h'@C + cb, staging, x8 cast."""
            h8w = h8[(t + 1) % 2][c]
            x8w = x8[(t + 1) % 2][c]
            s = t % SG
            g = t // SG
            if c == 0 and s == 0:
                stage[0] = stg.tile([P, 8, SG, AT], F16, tag="st", name="stage")
            pp = gt_ps.tile([P, 2, NCOL], F32, tag="g", name="pp")
            for b in range(2):
                nc.tensor.matmul(
                    pp[0:A, 0, :], cmat8[:, 2 * b : 2 * b + 2, 0:A],
                    h8w[:, 2 * b : 2 * b + 2, :],
                    start=(b == 0), stop=(b == 1), perf_mode=DR,
                )
            nc.scalar.activation(pred16[c][0:A, :], pp[0:A, 0, :],
                                 AF.Identity, bias=cb[:], scale=SCL)
            if t + 1 < T:
                nc.gpsimd.dma_start(x8w[0:A, 0, :], pred16[c][0:A, :])
            # batch-major transpose via DMA xbar straight into the stage
            for j in range(4):
                bt = c * 4 + j
                nc.sync.dma_start_transpose(
                    stage[0][:, bt, s, :], pred16[c][:AT, ts(j, P)]
                )
            if c == NB - 1 and s == SG - 1:
                for bt in range(8):
                    nc.sync.dma_start(
                        d_y[ts(bt, P), ts(g, SG), :],
                        stage[0][:, bt, :, :A],
                    )

        if bench:
            nc.sync.dma_start(d_dummy[:], cb[:])

        st = {}
        for rep in range(repeat):
            # prologue: step 0 gates
            for c in range(NB):
                mm1(0, c, 0)
                mm1(0, c, 1)
                mm2(0, c)
                tail(0, c)

            # steady state: emit step t's pred interleaved with step t+1's
            # gates, phase-split so every engine FIFO stays in ready order
            for t in range(T):
                last = t + 1 >= T
                for c in range(NB):
                    emit_pred(t, c)
                    if not last:
                        mm1(t + 1, c, 0)
                        mm1(t + 1, c, 1)
                        mm2(t + 1, c)
                        tail(t + 1, c)



_CACHE = {}


def _build(bench=False, repeat=1):
    key = f"nc_bench{repeat}" if bench else "nc"
    if key in _CACHE:
        return _CACHE[key]
    nc = bacc.Bacc(
        "TRN2",
        target_bir_lowering=False,
        debug=False,
        enable_asserts=False,
        num_devices=1 if bench else N_CORES,
    )
    _emit(nc, bench=bench, repeat=repeat)
    nc.compile()
    _CACHE[key] = nc
    return nc


def _hilo(vec8, s=8.0):
    """fp8 hi/lo split of a (x8-scaled) bias vector for const rows (8, 1/2)."""
    f64 = np.float64
    v = np.asarray(vec8, f64) / s
    assert np.abs(v).max() < 240.0
    hi = v.astype(NPF8).astype(f64)
    lo = (v - hi) * 16.0
    return hi, lo


def _prep_inputs(latent, fc1_w, fc1_b, fc2_w, fc2_b, W_ih, W_hh, b_ih, b_hh,
                 h1_w, h1_b, h2_w, h2_b):
    """Host-side weight fusion / fp8 layout prep. Returns per-core input maps."""
    f64 = np.float64

    C = h1_w.T.astype(f64) @ h2_w.T.astype(f64)                     # [H, A]
    c_b = h1_b.astype(f64) @ h2_w.T.astype(f64) + h2_b.astype(f64)  # [A]
    WihT = W_ih.T.astype(f64)                                       # [A, 3H]
    G_rz = W_hh.T[:, : 2 * H].astype(f64) + C @ WihT[:, : 2 * H]
    Whh_rz = W_hh.T[:, : 2 * H].astype(f64)
    Whh_n = W_hh.T[:, 2 * H :].astype(f64)
    Wih_n = WihT[:, 2 * H :]                                        # [A, H]
    x0row = np.full(A, -16.0); x0row[0] = 16.0
    b_rz = b_ih[: 2 * H].astype(f64) + b_hh[: 2 * H].astype(f64)
    b_rzf = b_rz + c_b @ WihT[:, : 2 * H]
    b_rz0 = b_rz + x0row @ WihT[:, : 2 * H]
    gxn0 = x0row @ Wih_n + b_ih[2 * H :].astype(f64)
    bihn = b_ih[2 * H :].astype(f64)
    bhhn = b_hh[2 * H :].astype(f64)
    Fmat = fc1_w.T.astype(f64) @ fc2_w.T.astype(f64)                # [128, H]
    fbv = fc1_b.astype(f64) @ fc2_w.T.astype(f64) + fc2_b.astype(f64)

    def drchunk(w, cols):
        # [H, cols] (pre-scaled) -> [P, HK, cols] fp8
        return np.ascontiguousarray(
            np.asarray(w, f64).reshape(HK, P, cols).transpose(1, 0, 2)
        ).astype(NPF8)

    def bias_blk(vec, cols):
        out = np.zeros((32, 2, cols), f64)
        hi, lo = _hilo(8.0 * np.asarray(vec, f64))
        out[0, 0, :] = hi
        out[0, 1, :] = lo
        return out.astype(NPF8)

    wihn8 = np.zeros((64, 4, H), f64)
    wihn8[0:A, 0, :] = 8.0 * Wih_n
    bh, bl = _hilo(8.0 * bihn)
    wihn8[0, 2, :] = bh
    wihn8[0, 3, :] = bl
    wihn0 = np.zeros((64, 4, H), f64)
    gh, gl = _hilo(8.0 * gxn0)
    wihn0[0, 2, :] = gh
    wihn0[0, 3, :] = gl

    common = {
        "hemat": np.ascontiguousarray(Fmat).astype(np.float16),
        "fb": np.ascontiguousarray(fbv.astype(np.float32).reshape(HK, P).T),
        "grz8": drchunk(8.0 * G_rz, 2 * H),
        "whhrz8": drchunk(8.0 * Whh_rz, 2 * H),
        "whhn8": drchunk(8.0 * Whh_n, H),
        "cmat8": drchunk(
            8.0 * np.concatenate([C, np.zeros((H, AT - A))], axis=1), AT),
        "wihn8": wihn8.astype(NPF8),
        "wihn0": wihn0.astype(NPF8),
        "brzf": bias_blk(b_rzf, 2 * H),
        "brz0": bias_blk(b_rz0, 2 * H),
        "bhhn": bias_blk(bhhn, H),
        "cb": c_b.astype(np.float32).reshape(A, 1),
    }
    in_maps = []
    for c in range(N_CORES):
        m = dict(common)
        m["latT"] = np.ascontiguousarray(
            latent[c * B : (c + 1) * B].T
        ).astype(np.float16)
        in_maps.append(m)
    return in_maps


def run(inputs, **kwargs):
    """Build (cached), run on 8 cores, return (y_full, BassKernelResults)."""
    nc = _build()
    in_maps = _prep_inputs(**inputs)
    res = run_bass_kernel_spmd(nc, in_maps, core_ids=list(range(N_CORES)), **kwargs)
    BF = inputs["latent"].shape[0]
    y = np.empty((BF, T + 1, A), np.float32)
    y[:, 0, :] = -16.0
    y[:, 0, 0] = 16.0
    for c in range(N_CORES):
        y[c * B : (c + 1) * B, 1:, :] = res.results[c]["y"].astype(np.float32)
    return y, res


def kernel(**inputs):
    inputs = {k: np.asarray(v) for k, v in inputs.items()}
    y, _ = run(inputs)
    return y


# revision 20
# speedup vs baseline: 1.0832x; 1.0832x over previous
"""Trainium2 Bass kernel for the GRU decoder problem.

Strategy (fp8 DoubleRow, 4-engine pipeline)
-------------------------------------------
Data-parallel over 8 NeuronCores: batch 8192 -> 1024 per core; weights
replicated.  Feature-major on-chip layout (features on SBUF partitions,
batch on the free dim); all recurrent matmuls run in fp8(e4m3) with
perf_mode=DoubleRow (256-contraction per instruction, 2x PE throughput).

Math (per core, B=1024, exact reformulation of the reference):
    h0   = (fc2 o fc1)(latent)            one fused [128,512] fp16 matmul
    per step t = 0..118:
        rz   = h @ G_rz + b               G_rz = Whh_rz + C@Wih_rz (t>=1)
        r,z  = sigmoid(rz)
        n    = tanh(x @ Wih_n + b_ihn + r*(h @ Whh_n + b_hhn))
        h    = n + z*(h - n)
        pred = h @ C + c_b                C = h1_w^T @ h2_w^T
        y[:, t+1] = pred;  x = pred

Precision / engine assignment (all verified rel_l2 ~ 6e-3 vs fp32):
  * fp8 e4m3 matmul operands, weights pre-scaled x8 so PSUM = 8x the
    true value; sigmoid/tanh/pred apply scale=2^-3.  h is kept in fp16
    (true scale) plus an fp8 matmul copy written directly by the DVE.
  * biases fold into the matmuls as fp8 hi/lo pairs (lo = 16x the hi
    rounding residual) against constant (8, 1/2) rows, so the
    two-feature-tile-merged sigmoids need no per-tile bias.
  * the "+ gxn" add of the n-gate runs on the Tensor engine: an fp16
    identity matmul accumulates u = r*(h@Whh_n+b) into the gxn PSUM
    bank, so tanh reads one PSUM tile and the DVE saves a full-width
    PSUM-rate op per pair.
  * elementwise is plain tensor_tensor (DVE 2x fp16 mode; the
    scalar_tensor_tensor form has NO fast mode so scales are folded
    into activation-scale/weights instead).  Per-pair chains: sigma
    order (r01, z01, r23, z23) lets pair0's d/e/h8' finish early and
    unlock the next step's block-0 matmuls.
  * the fp16 h' copy (only needed by the NEXT step's d) is computed
    on GpSimd, off the recurrence path; the x -> fp8 copy uses a
    gpsimd casting DMA; y is staged and stored as fp16 (host upcasts)
    via SP-engine HWDGE so the Pool queue stays short.
"""

import sys

import numpy as np
import ml_dtypes

if "/opt/trn_rl_repo" not in sys.path:
    sys.path.insert(0, "/opt/trn_rl_repo")

from contextlib import ExitStack

import concourse.bass as bass
import concourse.tile as tile
from concourse import bacc, mybir
from concourse.bass import ds, ts
from concourse.bass_utils import run_bass_kernel_spmd
from concourse.masks import make_identity

P = 128
H = 512
HK = 4            # H // 128 feature tiles
A = 40            # alphabet
AT = 48           # transpose-padded alphabet rows
B = 1024          # batch per core
NCOL = 512        # batch chunk (matmul N / PSUM bank)
NB = 2            # chunks per core
T = 119           # recurrent steps
SG = 17           # steps per output DMA group
NG = 7            # groups (7*17 = 119)
N_CORES = 8

SCL = 0.125       # PSUM descale (weights x8)

F16 = mybir.dt.float16
F32 = mybir.dt.float32
F8 = mybir.dt.float8e4
DR = mybir.MatmulPerfMode.DoubleRow
NPF8 = ml_dtypes.float8_e4m3

AF = mybir.ActivationFunctionType
OP = mybir.AluOpType


def _emit(nc, bench=False, repeat=1):
    # ---- DRAM I/O ------------------------------------------------------
    d_latT = nc.dram_tensor("latT", [P, B], F16, kind="ExternalInput").ap()
    d_hemat = nc.dram_tensor("hemat", [P, H], F16, kind="ExternalInput").ap()
    d_fb = nc.dram_tensor("fb", [P, HK], F32, kind="ExternalInput").ap()
    d_grz8 = nc.dram_tensor("grz8", [P, HK, 2 * H], F8, kind="ExternalInput").ap()
    d_whhrz8 = nc.dram_tensor("whhrz8", [P, HK, 2 * H], F8, kind="ExternalInput").ap()
    d_whhn8 = nc.dram_tensor("whhn8", [P, HK, H], F8, kind="ExternalInput").ap()
    d_cmat8 = nc.dram_tensor("cmat8", [P, HK, AT], F8, kind="ExternalInput").ap()
    d_wihn8 = nc.dram_tensor("wihn8", [64, 4, H], F8, kind="ExternalInput").ap()
    d_wihn0 = nc.dram_tensor("wihn0", [64, 4, H], F8, kind="ExternalInput").ap()
    d_brzf = nc.dram_tensor("brzf", [32, 2, 2 * H], F8, kind="ExternalInput").ap()
    d_brz0 = nc.dram_tensor("brz0", [32, 2, 2 * H], F8, kind="ExternalInput").ap()
    d_bhhn = nc.dram_tensor("bhhn", [32, 2, H], F8, kind="ExternalInput").ap()
    d_cb = nc.dram_tensor("cb", [A, 1], F32, kind="ExternalInput").ap()
    if bench:
        d_y = nc.dram_tensor("ybench", [B, T, A], F16, kind="Internal").ap()
        d_dummy = nc.dram_tensor("y", [A, 1], F32, kind="ExternalOutput").ap()
    else:
        d_y = nc.dram_tensor("y", [B, T, A], F16, kind="ExternalOutput").ap()

    with tile.TileContext(nc) as tc, ExitStack() as ctx:
        const = ctx.enter_context(tc.tile_pool(name="const", bufs=1))
        state = ctx.enter_context(tc.tile_pool(name="state", bufs=1))
        ew = ctx.enter_context(tc.tile_pool(name="ew", bufs=2))
        stg = ctx.enter_context(tc.tile_pool(name="stg", bufs=2))
        # PSUM: rz pairs 2x2 banks + gates/pred pairs 2x2 banks = 8 banks
        rz_ps = ctx.enter_context(tc.tile_pool(name="rz_ps", bufs=2, space="PSUM"))
        gt_ps = ctx.enter_context(tc.tile_pool(name="gt_ps", bufs=2, space="PSUM"))

        # ---- load constants/weights ------------------------------------
        def cload(name, dram, shape, dtype):
            t = const.tile(shape, dtype, tag=name, name=name)
            nc.sync.dma_start(t[:], dram)
            return t

        latT = cload("latT", d_latT, [P, B], F16)
        hemat = cload("hemat", d_hemat, [P, H], F16)
        fb = cload("fb", d_fb, [P, HK], F32)
        grz8 = cload("grz8", d_grz8, [P, HK, 2 * H], F8)
        whhrz8 = cload("whhrz8", d_whhrz8, [P, HK, 2 * H], F8)
        whhn8 = cload("whhn8", d_whhn8, [P, HK, H], F8)
        cmat8 = cload("cmat8", d_cmat8, [P, HK, AT], F8)
        wihn8 = cload("wihn8", d_wihn8, [64, 4, H], F8)
        wihn0 = cload("wihn0", d_wihn0, [64, 4, H], F8)
        brzf = cload("brzf", d_brzf, [32, 2, 2 * H], F8)
        brz0 = cload("brz0", d_brz0, [32, 2, 2 * H], F8)
        bhhn = cload("bhhn", d_bhhn, [32, 2, H], F8)
        cb = cload("cb", d_cb, [A, 1], F32)

        # constant rows for the bias matmuls: partition 0 = (8, 1/2)
        ones32 = const.tile([32, 2, NCOL], F8, tag="ones32", name="ones32")
        nc.gpsimd.memset(ones32[:], 0.0)
        nc.gpsimd.memset(ones32[0:1, 0, :], 8.0)
        nc.gpsimd.memset(ones32[0:1, 1, :], 0.5)

        # fp16 identity for the PE-side "accumulate u into gxn PSUM" add
        ident = const.tile([P, P], F16, tag="ident", name="ident")
        make_identity(nc, ident[:])

        # ---- persistent state ------------------------------------------
        h16 = [
            [state.tile([P, HK, NCOL], F16, tag=f"h16_{p}{c}", name=f"h16_{p}{c}")
             for c in range(NB)]
            for p in range(2)
        ]
        h8 = [
            [state.tile([P, HK, NCOL], F8, tag=f"h8_{p}{c}", name=f"h8_{p}{c}")
             for c in range(NB)]
            for p in range(2)
        ]
        # x8 [64, 4, NCOL]: slots 0,1 = x data (rows 0..39 = pred, cast
        # per step); slots 2,3 = (8, 1/2) const rows on partition 0 for
        # the b_ihn / gxn0 bias folds.
        x8 = [
            [state.tile([64, 4, NCOL], F8, tag=f"x8_{p}{c}", name=f"x8_{p}{c}")
             for c in range(NB)]
            for p in range(2)
        ]
        for p in range(2):
            for c in range(NB):
                nc.gpsimd.memset(x8[p][c][:], 0.0)
                nc.gpsimd.memset(x8[p][c][0:1, 2, :], 8.0)
                nc.gpsimd.memset(x8[p][c][0:1, 3, :], 0.5)
        pred16 = [state.tile([P, NCOL], F16, tag=f"pr{c}", name=f"pr{c}")
                  for c in range(NB)]
        for c in range(NB):
            nc.gpsimd.memset(pred16[c][:], 0.0)

        # ---- h0 = fused-fc(latent), fp16 -------------------------------
        for c in range(NB):
            for pair in range(2):
                pt = gt_ps.tile([P, 2, NCOL], F32, tag="g", name="h0ps")
                for i in range(2):
                    m = 2 * pair + i
                    nc.tensor.matmul(
                        pt[:, i, :], hemat[:, ts(m, P)], latT[:, ts(c, NCOL)],
                        start=True, stop=True,
                    )
                for i in range(2):
                    m = 2 * pair + i
                    nc.scalar.activation(
                        h16[0][c][:, m, :], pt[:, i, :], AF.Identity,
                        bias=fb[:, m : m + 1],
                    )
            nc.gpsimd.dma_start(h8[0][c][:], h16[0][c][:])

        stage = [None]

        def mm1(t, c, half):
            """rz DR matmuls + sigmoids for sigma-pairs [2*half, 2*half+1].

            sigma order over halves: (r01, z01) then (r23, z23), so each
            pair's r and z are both ready early for its elementwise chain.
            """
            par = t % 2
            h8r = h8[par][c]
            w_rz = grz8 if t else whhrz8
            b_blk = brzf if t else brz0
            if half == 0:
                st[(c, "r")] = ew.tile([P, HK, NCOL], F16, tag=f"r{c}", name="r")
                st[(c, "z")] = ew.tile([P, HK, NCOL], F16, tag=f"z{c}", name="z")
            for gate, pair in ((0, half), (1, half)):
                pt = rz_ps.tile([P, 2, NCOL], F32, tag="rz", name="rz")
                for i in range(2):
                    mp = 4 * gate + 2 * pair + i
                    for b in range(2):
                        nc.tensor.matmul(
                            pt[:, i, :],
                            w_rz[:, 2 * b : 2 * b + 2, ts(mp, P)],
                            h8r[:, 2 * b : 2 * b + 2, :],
                            start=(b == 0), stop=False, perf_mode=DR,
                        )
                    nc.tensor.matmul(
                        pt[:, i, :], b_blk[:, :, ts(mp, P)], ones32[:],
                        start=False, stop=True, perf_mode=DR,
                    )
                dst = st[(c, "r")] if gate == 0 else st[(c, "z")]
                q = pair * 2
                nc.scalar.activation(dst[:, q : q + 2, :], pt[:], AF.Sigmoid,
                                     scale=SCL)

        def mm2(t, c):
            """ghn/gxn DR matmuls + u = r*ghn (DVE)."""
            par = t % 2
            h8r = h8[par][c]
            x8r = x8[par][c]
            w_xn = wihn8 if t else wihn0
            r16 = st[(c, "r")]
            u16 = ew.tile([P, HK, NCOL], F16, tag=f"u{c}", name="u")
            pxs = []
            for pair in range(2):
                pg = gt_ps.tile([P, 2, NCOL], F32, tag="g", name="pg")
                for i in range(2):
                    m = 2 * pair + i
                    for b in range(2):
                        nc.tensor.matmul(
                            pg[:, i, :],
                            whhn8[:, 2 * b : 2 * b + 2, ts(m, P)],
                            h8r[:, 2 * b : 2 * b + 2, :],
                            start=(b == 0), stop=False, perf_mode=DR,
                        )
                    nc.tensor.matmul(
                        pg[:, i, :], bhhn[:, :, ts(m, P)], ones32[:],
                        start=False, stop=True, perf_mode=DR,
                    )
                px = rz_ps.tile([P, 2, NCOL], F32, tag="rz", name="px")
                for i in range(2):
                    m = 2 * pair + i
                    nc.tensor.matmul(
                        px[:, i, :], w_xn[:, 0:2, ts(m, P)], x8r[:, 0:2, :],
                        start=True, stop=False, perf_mode=DR,
                    )
                    nc.tensor.matmul(
                        px[:, i, :], w_xn[:, 2:4, ts(m, P)], x8r[:, 2:4, :],
                        start=False, stop=False, perf_mode=DR,
                    )
                pxs.append(px)
                q = 2 * pair
                nc.vector.tensor_tensor(
                    u16[:, q : q + 2, :], pg[:], r16[:, q : q + 2, :], OP.mult)
            st[(c, "u")] = u16
            st[(c, "px")] = pxs

        def tail(t, c):
            """identity-accumulate + tanh + h update (pair-wise chains)."""
            par = t % 2
            h16r = h16[par][c]
            h8w = h8[1 - par][c]
            h16w = h16[1 - par][c]
            u16 = st[(c, "u")]
            z16 = st[(c, "z")]
            pxs = st[(c, "px")]
            nt = ew.tile([P, HK, NCOL], F16, tag=f"n{c}", name="n")
            d16 = ew.tile([P, HK, NCOL], F16, tag=f"d{c}", name="d")
            e16 = ew.tile([P, HK, NCOL], F16, tag=f"e{c}", name="e")
            for pair in range(2):
                px = pxs[pair]
                q = 2 * pair
                sl = (slice(None), slice(q, q + 2), slice(None))
                for i in range(2):
                    nc.tensor.matmul(
                        px[:, i, :], ident[:], u16[:, q + i, :],
                        start=False, stop=(i == 1),
                    )
                nc.scalar.activation(nt[sl], px[:], AF.Tanh, scale=SCL)
                nc.vector.tensor_tensor(d16[sl], h16r[sl], nt[sl], OP.subtract)
                nc.vector.tensor_tensor(e16[sl], d16[sl], z16[sl], OP.mult)
                # fp8 h' for the next step's matmuls, straight from DVE
                nc.vector.tensor_tensor(h8w[sl], nt[sl], e16[sl], OP.add)
            st[(c, "nt")] = nt
            st[(c, "e")] = e16

        def emit_pred(t, c):
            """h16' (GpSimd, off-path), pred = h'@C + cb, staging, x8 cast."""
            h8w = h8[(t + 1) % 2][c]
            x8w = x8[(t + 1) % 2][c]
            nc.gpsimd.tensor_tensor(
                h16[(t + 1) % 2][c][:], st[(c, "nt")][:], st[(c, "e")][:],
                OP.add)
            s = t % SG
            g = t // SG
            if c == 0 and s == 0:
                stage[0] = stg.tile([P, 8, SG, AT], F16, tag="st", name="stage")
            pp = gt_ps.tile([P, 2, NCOL], F32, tag="g", name="pp")
            for b in range(2):
                nc.tensor.matmul(
                    pp[0:A, 0, :], cmat8[:, 2 * b : 2 * b + 2, 0:A],
                    h8w[:, 2 * b : 2 * b + 2, :],
                    start=(b == 0), stop=(b == 1), perf_mode=DR,
                )
            nc.scalar.activation(pred16[c][0:A, :], pp[0:A, 0, :],
                                 AF.Identity, bias=cb[:], scale=SCL)
            if t + 1 < T:
                nc.gpsimd.dma_start(x8w[0:A, 0, :], pred16[c][0:A, :])
            # batch-major transpose via DMA xbar straight into the stage
            for j in range(4):
                bt = c * 4 + j
                nc.sync.dma_start_transpose(
                    stage[0][:, bt, s, :], pred16[c][:AT, ts(j, P)]
                )
            if c == NB - 1 and s == SG - 1:
                for bt in range(8):
                    nc.sync.dma_start(
                        d_y[ts(bt, P), ts(g, SG), :],
                        stage[0][:, bt, :, :A],
                    )

        if bench:
            nc.sync.dma_start(d_dummy[:], cb[:])

        st = {}
        for rep in range(repeat):
            # prologue: step 0 gates
            for c in range(NB):
                mm1(0, c, 0)
                mm1(0, c, 1)
                mm2(0, c)
                tail(0, c)

            # steady state: emit step t's pred interleaved with step t+1's
            # gates, phase-split so every engine FIFO stays in ready order
            for t in range(T):
                last = t + 1 >= T
                for c in range(NB):
                    emit_pred(t, c)
                    if not last:
                        mm1(t + 1, c, 0)
                        mm1(t + 1, c, 1)
                        mm2(t + 1, c)
                        tail(t + 1, c)



_CACHE = {}


def _build(bench=False, repeat=1):
    key = f"nc_bench{repeat}" if bench else "nc"
    if key in _CACHE:
        return _CACHE[key]
    nc = bacc.Bacc(
        "TRN2",
        target_bir_lowering=False,
        debug=False,
        enable_asserts=False,
        num_devices=1 if bench else N_CORES,
    )
    _emit(nc, bench=bench, repeat=repeat)
    nc.compile()
    _CACHE[key] = nc
    return nc


def _hilo(vec8, s=8.0):
    """fp8 hi/lo split of a (x8-scaled) bias vector for const rows (8, 1/2)."""
    f64 = np.float64
    v = np.asarray(vec8, f64) / s
    assert np.abs(v).max() < 240.0
    hi = v.astype(NPF8).astype(f64)
    lo = (v - hi) * 16.0
    return hi, lo


def _prep_inputs(latent, fc1_w, fc1_b, fc2_w, fc2_b, W_ih, W_hh, b_ih, b_hh,
                 h1_w, h1_b, h2_w, h2_b):
    """Host-side weight fusion / fp8 layout prep. Returns per-core input maps."""
    f64 = np.float64

    C = h1_w.T.astype(f64) @ h2_w.T.astype(f64)                     # [H, A]
    c_b = h1_b.astype(f64) @ h2_w.T.astype(f64) + h2_b.astype(f64)  # [A]
    WihT = W_ih.T.astype(f64)                                       # [A, 3H]
    G_rz = W_hh.T[:, : 2 * H].astype(f64) + C @ WihT[:, : 2 * H]
    Whh_rz = W_hh.T[:, : 2 * H].astype(f64)
    Whh_n = W_hh.T[:, 2 * H :].astype(f64)
    Wih_n = WihT[:, 2 * H :]                                        # [A, H]
    x0row = np.full(A, -16.0); x0row[0] = 16.0
    b_rz = b_ih[: 2 * H].astype(f64) + b_hh[: 2 * H].astype(f64)
    b_rzf = b_rz + c_b @ WihT[:, : 2 * H]
    b_rz0 = b_rz + x0row @ WihT[:, : 2 * H]
    gxn0 = x0row @ Wih_n + b_ih[2 * H :].astype(f64)
    bihn = b_ih[2 * H :].astype(f64)
    bhhn = b_hh[2 * H :].astype(f64)
    Fmat = fc1_w.T.astype(f64) @ fc2_w.T.astype(f64)                # [128, H]
    fbv = fc1_b.astype(f64) @ fc2_w.T.astype(f64) + fc2_b.astype(f64)

    def drchunk(w, cols):
        # [H, cols] (pre-scaled) -> [P, HK, cols] fp8
        return np.ascontiguousarray(
            np.asarray(w, f64).reshape(HK, P, cols).transpose(1, 0, 2)
        ).astype(NPF8)

    def bias_blk(vec, cols):
        out = np.zeros((32, 2, cols), f64)
        hi, lo = _hilo(8.0 * np.asarray(vec, f64))
        out[0, 0, :] = hi
        out[0, 1, :] = lo
        return out.astype(NPF8)

    wihn8 = np.zeros((64, 4, H), f64)
    wihn8[0:A, 0, :] = 8.0 * Wih_n
    bh, bl = _hilo(8.0 * bihn)
    wihn8[0, 2, :] = bh
    wihn8[0, 3, :] = bl
    wihn0 = np.zeros((64, 4, H), f64)
    gh, gl = _hilo(8.0 * gxn0)
    wihn0[0, 2, :] = gh
    wihn0[0, 3, :] = gl

    common = {
        "hemat": np.ascontiguousarray(Fmat).astype(np.float16),
        "fb": np.ascontiguousarray(fbv.astype(np.float32).reshape(HK, P).T),
        "grz8": drchunk(8.0 * G_rz, 2 * H),
        "whhrz8": drchunk(8.0 * Whh_rz, 2 * H),
        "whhn8": drchunk(8.0 * Whh_n, H),
        "cmat8": drchunk(
            8.0 * np.concatenate([C, np.zeros((H, AT - A))], axis=1), AT),
        "wihn8": wihn8.astype(NPF8),
        "wihn0": wihn0.astype(NPF8),
        "brzf": bias_blk(b_rzf, 2 * H),
        "brz0": bias_blk(b_rz0, 2 * H),
        "bhhn": bias_blk(bhhn, H),
        "cb": c_b.astype(np.float32).reshape(A, 1),
    }
    in_maps = []
    for c in range(N_CORES):
        m = dict(common)
        m["latT"] = np.ascontiguousarray(
            latent[c * B : (c + 1) * B].T
        ).astype(np.float16)
        in_maps.append(m)
    return in_maps


def run(inputs, **kwargs):
    """Build (cached), run on 8 cores, return (y_full, BassKernelResults)."""
    nc = _build()
    in_maps = _prep_inputs(**inputs)
    res = run_bass_kernel_spmd(nc, in_maps, core_ids=list(range(N_CORES)), **kwargs)
    BF = inputs["latent"].shape[0]
    y = np.empty((BF, T + 1, A), np.float32)
    y[:, 0, :] = -16.0
    y[:, 0, 0] = 16.0
    for c in range(N_CORES):
        y[c * B : (c + 1) * B, 1:, :] = res.results[c]["y"].astype(np.float32)
    return y, res


def kernel(**inputs):
    inputs = {k: np.asarray(v) for k, v in inputs.items()}
    y, _ = run(inputs)
    return y
